# revision 5
# baseline (speedup 1.0000x reference)
"""8-core Trainium2 Bass kernel for nn_KahlerAttention.

Strategy:
- Each QuaternionLinear is reformulated as a dense [1024,1024] matmul with a
  sign-structured block weight matrix built on the host from the four
  [256,256] sub-weights (pure weight preprocessing).
- Sharding: core c = 4*b + g handles batch b and head group g (heads
  {g, g+4, g+8, g+12} = the 4 quaternion components of d4-positions
  [64g, 64g+64)).  QKV projections are column-sharded (each core computes
  only its 256 channels), quat_norm is local to a core, attention for its 4
  heads is local, then an AllGather over the 4 cores of a batch group
  rebuilds the full attention output and the output projection is
  column-sharded.
- The softmax bias mean(sin(J[:T]))*0.1 is a constant added to every score;
  softmax is shift-invariant, so it is dropped.
- Compute dtype bf16 (f32 PSUM accumulation); scores are bounded by
  |q_h||k_h|*scale <= 2, so no max-subtraction pass is needed.
- Row-sums of exp(scores) come free from the AV matmul via an appended
  ones-column on V (M=65).
"""
import numpy as np
import concourse.bass as bass
import concourse.mybir as mybir
import concourse.tile as tile
import concourse.bacc as bacc
from concourse.bass_utils import run_bass_kernel_spmd

F32 = mybir.dt.float32
BF16 = mybir.dt.bfloat16
AF = mybir.ActivationFunctionType
ALU = mybir.AluOpType

B, T, D = 2, 2048, 1024
D4, H, Dh = 256, 16, 64
NG = 4            # head groups = cores per batch
NCORES = 8
ND = D // 128     # 8 D-tiles
NTB = T // 512    # 4 t-blocks
NST = T // 128    # 16 s-tiles
EPS = 1e-6

_SIGN_TABLE = [
    [(0, +1), (1, -1), (2, -1), (3, -1)],
    [(1, +1), (0, +1), (3, -1), (2, +1)],
    [(2, +1), (3, +1), (0, +1), (1, -1)],
    [(3, +1), (2, -1), (1, +1), (0, +1)],
]
_BSIGNS = [
    [+1, -1, -1, -1],
    [+1, +1, +1, -1],
    [+1, -1, +1, +1],
    [+1, +1, -1, +1],
]


def _build_big(W, b):
    W = np.asarray(W, np.float32)
    b = np.asarray(b, np.float32)
    Wbig = np.zeros((D, D), np.float32)
    bbig = np.zeros(D, np.float32)
    for o in range(4):
        for c in range(4):
            m, s = _SIGN_TABLE[o][c]
            Wbig[c * D4:(c + 1) * D4, o * D4:(o + 1) * D4] = s * W[m]
        bbig[o * D4:(o + 1) * D4] = sum(
            sg * b[w] for w, sg in zip(range(4), _BSIGNS[o]))
    return Wbig, bbig


def _cols_for_group(g):
    return np.array([256 * w + 64 * g + j for w in range(4) for j in range(64)])


DEBUG = False


def _build_nc():
    nc = bacc.Bacc("TRN2", target_bir_lowering=False, debug=False,
                   num_devices=NCORES)
    xt_d = nc.dram_tensor("xt", [D, T], F32, kind="ExternalInput")
    wq_d = nc.dram_tensor("wq", [D, 256], F32, kind="ExternalInput")
    wk_d = nc.dram_tensor("wk", [D, 256], F32, kind="ExternalInput")
    wv_d = nc.dram_tensor("wv", [D, 260], F32, kind="ExternalInput")
    wo_d = nc.dram_tensor("wo", [D, 256], F32, kind="ExternalInput")
    bq_d = nc.dram_tensor("bq", [128, 2], F32, kind="ExternalInput")
    bk_d = nc.dram_tensor("bk", [128, 2], F32, kind="ExternalInput")
    bv_d = nc.dram_tensor("bv", [128, 260], F32, kind="ExternalInput")
    bo_d = nc.dram_tensor("bo", [128, 2], F32, kind="ExternalInput")
    s0_d = nc.dram_tensor("s0", [128, 128], F32, kind="ExternalInput")
    out_d = nc.dram_tensor("out", [256, T], F32, kind="ExternalOutput")
    if DEBUG:
        dbg_qt = nc.dram_tensor("dbg_qt", [128, T], F32, kind="ExternalOutput")
        dbg_kt = nc.dram_tensor("dbg_kt", [128, T], F32, kind="ExternalOutput")
        dbg_v = nc.dram_tensor("dbg_v", [128, 260], F32, kind="ExternalOutput")
        dbg_e = nc.dram_tensor("dbg_e", [128, 1024], F32, kind="ExternalOutput")
        dbg_ao = nc.dram_tensor("dbg_ao", [128, T], F32, kind="ExternalOutput")
        dbg_ag = nc.dram_tensor("dbg_ag", [128, T], F32, kind="ExternalOutput")

    with tile.TileContext(nc) as tc:
        with (
            tc.tile_pool(name="pers", bufs=1) as pers,
            tc.tile_pool(name="xa", bufs=8) as xa,
            tc.tile_pool(name="ld", bufs=2) as ld,
            tc.tile_pool(name="work", bufs=3) as work,
            tc.tile_pool(name="ep", bufs=3) as ep_pool,
            tc.tile_pool(name="ps", bufs=2, space="PSUM") as ps,
            tc.tile_pool(name="psa", bufs=2, space="PSUM") as psa,
            tc.tile_pool(name="dram", bufs=1, space="DRAM") as dram,
        ):
            # ---- load & cast inputs to bf16
            def load_cast(dram_ap, shape, tag, pool=pers):
                f = ld.tile(shape, F32, tag="ldf", name="ldf")
                nc.sync.dma_start(f[:], dram_ap)
                t = pool.tile(shape, BF16, tag=tag, name=tag)
                nc.vector.tensor_copy(t[:], f[:])
                return t

            xbf = [load_cast(xt_d[128 * i:128 * (i + 1), :], [128, T], "xa", xa)
                   for i in range(ND)]
            wqb = [load_cast(wq_d[128 * i:128 * (i + 1), :], [128, 256], f"wq{i}")
                   for i in range(ND)]
            wkb = [load_cast(wk_d[128 * i:128 * (i + 1), :], [128, 256], f"wk{i}")
                   for i in range(ND)]
            wvb = [load_cast(wv_d[128 * i:128 * (i + 1), :], [128, 260], f"wv{i}")
                   for i in range(ND)]
            wob = [load_cast(wo_d[128 * i:128 * (i + 1), :], [128, 256], f"wo{i}")
                   for i in range(ND)]
            s0b = load_cast(s0_d[:], [128, 128], "s0")
            bq_sb = pers.tile([128, 2], F32, tag="bq")
            nc.sync.dma_start(bq_sb[:], bq_d[:])
            bk_sb = pers.tile([128, 2], F32, tag="bk")
            nc.sync.dma_start(bk_sb[:], bk_d[:])
            bv_sb = pers.tile([128, 260], F32, tag="bv")
            nc.sync.dma_start(bv_sb[:], bv_d[:])
            bo_sb = pers.tile([128, 2], F32, tag="bo")
            nc.sync.dma_start(bo_sb[:], bo_d[:])

            # ---- Q^T / K^T projections ([channel, t] layout) + quat norm
            QT = [pers.tile([128, T], BF16, tag=f"QT{ct}", name=f"QT{ct}") for ct in range(2)]
            KT = [pers.tile([128, T], BF16, tag=f"KT{ct}", name=f"KT{ct}") for ct in range(2)]
            for wb, bias_sb, dst, sc in ((wqb, bq_sb, QT, 1.0 / 64.0),
                                         (wkb, bk_sb, KT, 1.0)):
                for tb in range(NTB):
                    ts = bass.ts(tb, 512)
                    raws, sqs = [], []
                    for ct in range(2):
                        pq = ps.tile([128, 512], F32, tag="mm", name="pq")
                        for dt in range(ND):
                            nc.tensor.matmul(
                                pq[:], wb[dt][:, 128 * ct:128 * (ct + 1)],
                                xbf[dt][:, ts],
                                start=(dt == 0), stop=(dt == ND - 1))
                        raw_t = work.tile([128, 512], BF16, tag=f"raw{ct}")
                        nc.vector.tensor_scalar_add(
                            raw_t[:], pq[:], bias_sb[:, ct:ct + 1])
                        sq_t = work.tile([128, 512], BF16, tag=f"sq{ct}")
                        nc.vector.tensor_tensor(
                            sq_t[:], raw_t[:], raw_t[:], ALU.mult)
                        raws.append(raw_t)
                        sqs.append(sq_t)
                    pn = ps.tile([128, 512], F32, tag="mm", name="pn")
                    nc.tensor.matmul(pn[:], s0b[:], sqs[0][:],
                                     start=True, stop=False)
                    nc.tensor.matmul(pn[:], s0b[:], sqs[1][:],
                                     start=False, stop=True)
                    n2e = work.tile([128, 512], F32, tag="n2e")
                    nc.vector.tensor_scalar_add(n2e[:], pn[:], EPS)
                    rcpn = work.tile([128, 512], F32, tag="rcpn")
                    nc.vector.reciprocal_approx_fast(rcpn[:], n2e[:])
                    rn = work.tile([128, 512], BF16, tag="rn")
                    nc.scalar.activation(rn[:], rcpn[:], AF.Sqrt, scale=sc)
                    for ct in range(2):
                        nc.vector.tensor_tensor(
                            dst[ct][:, ts], raws[ct][:], rn[:], ALU.mult)

            # ---- V projection (natural [t, channel-ext] layout, 4x(64+1) cols)
            V3 = pers.tile([128, NST, 260], BF16, tag="V3")
            for tt in range(NST):
                pv = ps.tile([128, 260], F32, tag="mm", name="pv")
                for dt in range(ND):
                    nc.tensor.matmul(pv[:], xbf[dt][:, bass.ts(tt, 128)],
                                     wvb[dt][:],
                                     start=(dt == 0), stop=(dt == ND - 1))
                nc.vector.tensor_tensor(V3[:, tt, :], pv[:], bv_sb[:], ALU.add)

            # ---- attention: 2 head pairs x 4 t-blocks x 16 s-tiles
            aoT = [pers.tile([128, T], BF16, tag=f"ao{ct}", name=f"ao{ct}") for ct in range(2)]
            for hp in range(2):
                for tb in range(NTB):
                    ts = bass.ts(tb, 512)
                    avt = [psa.tile([65, 512], F32, tag=f"avt{sl}", name=f"avt{sl}")
                           for sl in range(2)]
                    for st in range(NST):
                        ss = bass.ts(st, 128)
                        stp = ps.tile([128, 1024], F32, tag="mm", name="stp")
                        nc.tensor.matmul(stp[:, 0:512],
                                         KT[hp][0:64, ss], QT[hp][0:64, ts],
                                         start=True, stop=True,
                                         tile_position=(0, 0))
                        nc.tensor.matmul(stp[:, 512:1024],
                                         KT[hp][64:128, ss], QT[hp][64:128, ts],
                                         start=True, stop=True,
                                         tile_position=(64, 0))
                        e_t = ep_pool.tile([128, 1024], BF16, tag="e")
                        nc.scalar.activation(e_t[:], stp[:], AF.Exp)
                        if DEBUG and hp == 0 and tb == 0 and st == 0:
                            de_f = work.tile([128, 1024], F32, tag="dbge", name="dbge")
                            nc.vector.tensor_copy(de_f[:], e_t[:])
                            nc.sync.dma_start(dbg_e[:], de_f[:])
                        for sl in range(2):
                            w = 2 * hp + sl
                            nc.tensor.matmul(
                                avt[sl][:], V3[:, st, 65 * w:65 * w + 65],
                                e_t[:, 512 * sl:512 * (sl + 1)],
                                start=(st == 0), stop=(st == NST - 1))
                    for sl in range(2):
                        w = 2 * hp + sl
                        rsum = work.tile([1, 512], F32, tag="rsum")
                        nc.vector.tensor_copy(rsum[:], avt[sl][64:65, :])
                        rcp = work.tile([1, 512], F32, tag="rcp")
                        nc.vector.reciprocal_approx_fast(rcp[:], rsum[:])
                        bc = work.tile([64, 512], F32, tag="bc")
                        nc.gpsimd.partition_broadcast(bc[:], rcp[0:1, :])
                        p0 = (w % 2) * 64
                        nc.vector.tensor_tensor(
                            aoT[hp][p0:p0 + 64, ts], avt[sl][0:64, :], bc[:],
                            ALU.mult)

            if DEBUG:
                for nm, tl in (("dbg_qt", QT[0]), ("dbg_kt", KT[0]),
                               ("dbg_ao", aoT[0])):
                    dt_f = work.tile([128, T], F32, tag="dbgf", name="dbgf")
                    nc.vector.tensor_copy(dt_f[:], tl[:])
                    nc.sync.dma_start({"dbg_qt": dbg_qt, "dbg_kt": dbg_kt,
                                       "dbg_ao": dbg_ao}[nm][:], dt_f[:])
                dt_f = work.tile([128, 260], F32, tag="dbgv", name="dbgv")
                nc.vector.tensor_copy(dt_f[:], V3[:, 0, :])
                nc.sync.dma_start(dbg_v[:], dt_f[:])

            # ---- AllGather attention outputs within each batch group
            agi = dram.tile([256, T], BF16)
            ago = dram.tile([4 * 256, T], BF16)
            for ct in range(2):
                nc.sync.dma_start(agi[128 * ct:128 * (ct + 1), :], aoT[ct][:])
            nc.gpsimd.collective_compute(
                "AllGather", ALU.bypass,
                replica_groups=[[0, 1, 2, 3], [4, 5, 6, 7]],
                ins=[agi[:].opt()], outs=[ago[:].opt()])
            aog = [xa.tile([128, T], BF16, tag="xa", name="aog") for _ in range(ND)]
            for i in range(ND):
                nc.sync.dma_start(aog[i][:], ago[128 * i:128 * (i + 1), :])
            if DEBUG:
                dg_f = work.tile([128, T], F32, tag="dbgf", name="dbgf")
                nc.vector.tensor_copy(dg_f[:], aog[0][:])
                nc.sync.dma_start(dbg_ag[:], dg_f[:])

            # ---- output projection (column slice of 256 channels)
            for ocb in range(2):
                outp = pers.tile([128, T], F32, tag=f"outp{ocb}")
                for tb in range(NTB):
                    ts = bass.ts(tb, 512)
                    po = ps.tile([128, 512], F32, tag="mm", name="po")
                    for ci in range(ND):
                        nc.tensor.matmul(po[:],
                                         wob[ci][:, 128 * ocb:128 * (ocb + 1)],
                                         aog[ci][:, ts],
                                         start=(ci == 0), stop=(ci == ND - 1))
                    nc.vector.tensor_scalar_add(
                        outp[:, ts], po[:], bo_sb[:, ocb:ocb + 1])
                nc.sync.dma_start(out_d[128 * ocb:128 * (ocb + 1), :], outp[:])
    nc.compile()
    return nc


_NC_CACHE = {}


def get_nc():
    if "nc" not in _NC_CACHE:
        _NC_CACHE["nc"] = _build_nc()
    return _NC_CACHE["nc"]


def make_in_maps(x, qW, qb, kW, kb, vW, vb, oW, ob, J):
    x = np.asarray(x, np.float32)
    Wq, bq = _build_big(qW, qb)
    Wk, bk = _build_big(kW, kb)
    Wv, bv = _build_big(vW, vb)
    Wo, bo = _build_big(oW, ob)
    perm = np.concatenate([_cols_for_group(g) for g in range(NG)])
    Wo_perm = np.ascontiguousarray(Wo[perm, :])
    s0 = (np.arange(128)[:, None] % 64 == np.arange(128)[None, :] % 64)
    s0 = s0.astype(np.float32)
    xts = [np.ascontiguousarray(x[b].T) for b in range(B)]
    in_maps = []
    for c in range(NCORES):
        b, g = divmod(c, NG)
        cols = _cols_for_group(g)
        wv_ext = np.zeros((D, 260), np.float32)
        bv_ext = np.zeros(260, np.float32)
        for w in range(4):
            wv_ext[:, 65 * w:65 * w + 64] = Wv[:, cols[64 * w:64 * w + 64]]
            bv_ext[65 * w:65 * w + 64] = bv[cols[64 * w:64 * w + 64]]
            bv_ext[65 * w + 64] = 1.0
        in_maps.append({
            "xt": xts[b],
            "wq": np.ascontiguousarray(Wq[:, cols]),
            "wk": np.ascontiguousarray(Wk[:, cols]),
            "wv": wv_ext,
            "wo": np.ascontiguousarray(Wo_perm[:, 256 * g:256 * (g + 1)]),
            "bq": np.ascontiguousarray(bq[cols].reshape(2, 128).T),
            "bk": np.ascontiguousarray(bk[cols].reshape(2, 128).T),
            "bv": np.broadcast_to(bv_ext, (128, 260)).copy(),
            "bo": np.ascontiguousarray(
                bo[256 * g:256 * (g + 1)].reshape(2, 128).T),
            "s0": s0,
        })
    return in_maps


def assemble_output(results):
    out = np.zeros((B, T, D), np.float32)
    for c in range(NCORES):
        b, g = divmod(c, NG)
        out[b, :, 256 * g:256 * (g + 1)] = results[c]["out"].T
    return out


def kernel(**inputs):
    in_maps = make_in_maps(**inputs)
    nc = get_nc()
    res = run_bass_kernel_spmd(nc, in_maps, core_ids=list(range(NCORES)))
    return assemble_output(res.results)


if __name__ == "__main__":
    import reference
    inputs = {k: np.asarray(v) for k, v in reference.setup_inputs().items()}
    expected = np.asarray(reference.reference(**inputs))
    actual = kernel(**inputs)
    err = np.linalg.norm(actual - expected) / np.linalg.norm(expected)
    print(f"Relative error: {err:.3e}")


# revision 7
# speedup vs baseline: 23.6541x; 23.6541x over previous
"""8-core Trainium2 Bass kernel for nn_KahlerAttention.

Strategy:
- Each QuaternionLinear is reformulated as a dense [1024,1024] matmul with a
  sign-structured block weight matrix built on the host from the four
  [256,256] sub-weights (pure weight preprocessing).
- Sharding: core c = 4*b + g handles batch b and head group g (heads
  {g, g+4, g+8, g+12} = the 4 quaternion components of d4-positions
  [64g, 64g+64)).  QKV projections are column-sharded (each core computes
  only its 256 channels), quat_norm is local to a core, attention for its 4
  heads is local, then an AllGather over the 4 cores of a batch group
  rebuilds the full attention output and the output projection is
  column-sharded.
- The softmax bias mean(sin(J[:T]))*0.1 is a constant added to every score;
  softmax is shift-invariant, so it is dropped.
- Compute dtype bf16 (f32 PSUM accumulation); scores are bounded by
  |q_h||k_h|*scale <= 2, so no max-subtraction pass is needed.
- Row-sums of exp(scores) come free from the AV matmul via an appended
  ones-column on V (M=65).
"""
import numpy as np
import concourse.bass as bass
import concourse.mybir as mybir
import concourse.tile as tile
import concourse.bacc as bacc
from concourse.bass_utils import run_bass_kernel_spmd

F32 = mybir.dt.float32
BF16 = mybir.dt.bfloat16
AF = mybir.ActivationFunctionType
ALU = mybir.AluOpType

B, T, D = 2, 2048, 1024
D4, H, Dh = 256, 16, 64
NG = 4            # head groups = cores per batch
NCORES = 8
ND = D // 128     # 8 D-tiles
NTB = T // 512    # 4 t-blocks
NST = T // 128    # 16 s-tiles
EPS = 1e-6

_SIGN_TABLE = [
    [(0, +1), (1, -1), (2, -1), (3, -1)],
    [(1, +1), (0, +1), (3, -1), (2, +1)],
    [(2, +1), (3, +1), (0, +1), (1, -1)],
    [(3, +1), (2, -1), (1, +1), (0, +1)],
]
_BSIGNS = [
    [+1, -1, -1, -1],
    [+1, +1, +1, -1],
    [+1, -1, +1, +1],
    [+1, +1, -1, +1],
]


def _build_big(W, b):
    W = np.asarray(W, np.float32)
    b = np.asarray(b, np.float32)
    Wbig = np.zeros((D, D), np.float32)
    bbig = np.zeros(D, np.float32)
    for o in range(4):
        for c in range(4):
            m, s = _SIGN_TABLE[o][c]
            Wbig[c * D4:(c + 1) * D4, o * D4:(o + 1) * D4] = s * W[m]
        bbig[o * D4:(o + 1) * D4] = sum(
            sg * b[w] for w, sg in zip(range(4), _BSIGNS[o]))
    return Wbig, bbig


def _cols_for_group(g):
    return np.array([256 * w + 64 * g + j for w in range(4) for j in range(64)])


DEBUG = False


def _build_nc():
    nc = bacc.Bacc("TRN2", target_bir_lowering=False, debug=False,
                   num_devices=NCORES)
    xt_d = nc.dram_tensor("xt", [D, T], F32, kind="ExternalInput")
    wq_d = nc.dram_tensor("wq", [D, 256], F32, kind="ExternalInput")
    wk_d = nc.dram_tensor("wk", [D, 256], F32, kind="ExternalInput")
    wv_d = nc.dram_tensor("wv", [D, 260], F32, kind="ExternalInput")
    wo_d = nc.dram_tensor("wo", [D, 256], F32, kind="ExternalInput")
    bq_d = nc.dram_tensor("bq", [128, 2], F32, kind="ExternalInput")
    bk_d = nc.dram_tensor("bk", [128, 2], F32, kind="ExternalInput")
    bv_d = nc.dram_tensor("bv", [128, 260], F32, kind="ExternalInput")
    bo_d = nc.dram_tensor("bo", [128, 2], F32, kind="ExternalInput")
    s0_d = nc.dram_tensor("s0", [128, 128], F32, kind="ExternalInput")
    out_d = nc.dram_tensor("out", [256, T], F32, kind="ExternalOutput")
    if DEBUG:
        dbg_qt = nc.dram_tensor("dbg_qt", [128, T], F32, kind="ExternalOutput")
        dbg_kt = nc.dram_tensor("dbg_kt", [128, T], F32, kind="ExternalOutput")
        dbg_v = nc.dram_tensor("dbg_v", [128, 260], F32, kind="ExternalOutput")
        dbg_e = nc.dram_tensor("dbg_e", [128, 1024], F32, kind="ExternalOutput")
        dbg_ao = nc.dram_tensor("dbg_ao", [128, T], F32, kind="ExternalOutput")
        dbg_ag = nc.dram_tensor("dbg_ag", [128, T], F32, kind="ExternalOutput")

    with tile.TileContext(nc) as tc:
        with (
            tc.tile_pool(name="pers", bufs=1) as pers,
            tc.tile_pool(name="xa", bufs=8) as xa,
            tc.tile_pool(name="ld", bufs=2) as ld,
            tc.tile_pool(name="work", bufs=3) as work,
            tc.tile_pool(name="ep", bufs=4) as ep_pool,
            tc.tile_pool(name="ps", bufs=3, space="PSUM") as ps,
            tc.tile_pool(name="psa", bufs=1, space="PSUM") as psa,
            tc.tile_pool(name="dram", bufs=1, space="DRAM") as dram,
        ):
            # ---- load & cast inputs to bf16
            def load_cast(dram_ap, shape, tag, pool=pers):
                ltag = "ldf" if shape[1] >= 1024 else "ldw"
                f = ld.tile(shape, F32, tag=ltag, name=ltag, bufs=2 if ltag == "ldf" else 4)
                nc.sync.dma_start(f[:], dram_ap)
                t = pool.tile(shape, BF16, tag=tag, name=tag)
                nc.vector.tensor_copy(t[:], f[:])
                return t

            xbf = [load_cast(xt_d[128 * i:128 * (i + 1), :], [128, T], "xa", xa)
                   for i in range(ND)]
            wqb = [load_cast(wq_d[128 * i:128 * (i + 1), :], [128, 256], f"wq{i}")
                   for i in range(ND)]
            wkb = [load_cast(wk_d[128 * i:128 * (i + 1), :], [128, 256], f"wk{i}")
                   for i in range(ND)]
            wvb = [load_cast(wv_d[128 * i:128 * (i + 1), :], [128, 260], f"wv{i}")
                   for i in range(ND)]
            wob = [load_cast(wo_d[128 * i:128 * (i + 1), :], [128, 256], f"wo{i}")
                   for i in range(ND)]
            s0b = load_cast(s0_d[:], [128, 128], "s0")
            bq_sb = pers.tile([128, 2], F32, tag="bq")
            nc.sync.dma_start(bq_sb[:], bq_d[:])
            bk_sb = pers.tile([128, 2], F32, tag="bk")
            nc.sync.dma_start(bk_sb[:], bk_d[:])
            bv_sb = pers.tile([128, 260], F32, tag="bv")
            nc.sync.dma_start(bv_sb[:], bv_d[:])
            bo_sb = pers.tile([128, 2], F32, tag="bo")
            nc.sync.dma_start(bo_sb[:], bo_d[:])

            # ---- Q^T / K^T projections ([channel, t] layout) + quat norm
            QT = [pers.tile([128, T], BF16, tag=f"QT{ct}", name=f"QT{ct}") for ct in range(2)]
            KT = [pers.tile([128, T], BF16, tag=f"KT{ct}", name=f"KT{ct}") for ct in range(2)]
            for wb, bias_sb, dst, sc in ((wqb, bq_sb, QT, 1.0 / 64.0),
                                         (wkb, bk_sb, KT, 1.0)):
                for tb in range(NTB):
                    ts = bass.ts(tb, 512)
                    raws, sqs = [], []
                    for ct in range(2):
                        pq = ps.tile([128, 512], F32, tag="mm", name="pq")
                        for dt in range(ND):
                            nc.tensor.matmul(
                                pq[:], wb[dt][:, 128 * ct:128 * (ct + 1)],
                                xbf[dt][:, ts],
                                start=(dt == 0), stop=(dt == ND - 1))
                        raw_t = work.tile([128, 512], BF16, tag=f"raw{ct}")
                        nc.vector.tensor_scalar_add(
                            raw_t[:], pq[:], bias_sb[:, ct:ct + 1])
                        sq_t = work.tile([128, 512], BF16, tag=f"sq{ct}")
                        nc.vector.tensor_tensor(
                            sq_t[:], raw_t[:], raw_t[:], ALU.mult)
                        raws.append(raw_t)
                        sqs.append(sq_t)
                    pn = ps.tile([128, 512], F32, tag="mm", name="pn")
                    nc.tensor.matmul(pn[:], s0b[:], sqs[0][:],
                                     start=True, stop=False)
                    nc.tensor.matmul(pn[:], s0b[:], sqs[1][:],
                                     start=False, stop=True)
                    n2e = work.tile([128, 512], F32, tag="n2e")
                    nc.vector.tensor_scalar_add(n2e[:], pn[:], EPS)
                    rcpn = work.tile([128, 512], F32, tag="rcpn")
                    nc.vector.reciprocal_approx_fast(rcpn[:], n2e[:])
                    rn = work.tile([128, 512], BF16, tag="rn")
                    nc.scalar.activation(rn[:], rcpn[:], AF.Sqrt, scale=sc)
                    for ct in range(2):
                        nc.vector.tensor_tensor(
                            dst[ct][:, ts], raws[ct][:], rn[:], ALU.mult)

            # ---- V projection (natural [t, channel-ext] layout, 4x(64+1) cols)
            V3 = pers.tile([128, NST, 260], BF16, tag="V3")
            for tt in range(NST):
                pv = ps.tile([128, 260], F32, tag="mm", name="pv")
                for dt in range(ND):
                    nc.tensor.matmul(pv[:], xbf[dt][:, bass.ts(tt, 128)],
                                     wvb[dt][:],
                                     start=(dt == 0), stop=(dt == ND - 1))
                nc.vector.tensor_tensor(V3[:, tt, :], pv[:], bv_sb[:], ALU.add)

            # ---- attention: t-blocks outer so early t-halves can be
            # gathered + output-projected while later t-blocks compute
            aoT = [pers.tile([128, T], BF16, tag=f"ao{ct}", name=f"ao{ct}") for ct in range(2)]
            agi = [dram.tile([256, T // 2], BF16, name=f"agi{h}") for h in range(2)]
            ago = [dram.tile([4 * 256, T // 2], BF16, name=f"ago{h}") for h in range(2)]
            aog = [xa.tile([128, T], BF16, tag="xa", name="aog") for _ in range(ND)]
            outp = [pers.tile([128, T], F32, tag=f"outp{ocb}", name=f"outp{ocb}")
                    for ocb in range(2)]

            def emit_gather_oproj(half):
                # AllGather columns [1024*half, 1024*(half+1)) then project them
                hs = bass.ts(half, T // 2)
                for ct in range(2):
                    nc.sync.dma_start(agi[half][128 * ct:128 * (ct + 1), :],
                                      aoT[ct][:, hs])
                nc.gpsimd.collective_compute(
                    "AllGather", ALU.bypass,
                    replica_groups=[[0, 1, 2, 3], [4, 5, 6, 7]],
                    ins=[agi[half][:].opt()], outs=[ago[half][:].opt()])
                for i in range(ND):
                    nc.sync.dma_start(
                        aog[i][:, hs],
                        ago[half][128 * i:128 * (i + 1), :])
                for ocb in range(2):
                    for tb in (2 * half, 2 * half + 1):
                        ts = bass.ts(tb, 512)
                        po = ps.tile([128, 512], F32, tag="mm", name="po")
                        for ci in range(ND):
                            nc.tensor.matmul(
                                po[:], wob[ci][:, 128 * ocb:128 * (ocb + 1)],
                                aog[ci][:, ts],
                                start=(ci == 0), stop=(ci == ND - 1))
                        nc.vector.tensor_scalar_add(
                            outp[ocb][:, ts], po[:], bo_sb[:, ocb:ocb + 1])

            for tb in range(NTB):
                ts = bass.ts(tb, 512)
                for hp in range(2):
                    avt = [psa.tile([65, 512], F32, tag=f"avt{sl}", name=f"avt{sl}")
                           for sl in range(2)]
                    for st in range(NST):
                        ss = bass.ts(st, 128)
                        stp = ps.tile([128, 1024], F32, tag="mm", name="stp")
                        nc.tensor.matmul(stp[:, 0:512],
                                         KT[hp][0:64, ss], QT[hp][0:64, ts],
                                         start=True, stop=True,
                                         tile_position=(0, 0))
                        nc.tensor.matmul(stp[:, 512:1024],
                                         KT[hp][64:128, ss], QT[hp][64:128, ts],
                                         start=True, stop=True,
                                         tile_position=(64, 0))
                        e_t = ep_pool.tile([128, 1024], BF16, tag="e")
                        nc.scalar.activation(e_t[:], stp[:], AF.Exp)
                        if DEBUG and hp == 0 and tb == 0 and st == 0:
                            de_f = work.tile([128, 1024], F32, tag="dbge", name="dbge")
                            nc.vector.tensor_copy(de_f[:], e_t[:])
                            nc.sync.dma_start(dbg_e[:], de_f[:])
                        for sl in range(2):
                            w = 2 * hp + sl
                            nc.tensor.matmul(
                                avt[sl][:], V3[:, st, 65 * w:65 * w + 65],
                                e_t[:, 512 * sl:512 * (sl + 1)],
                                start=(st == 0), stop=(st == NST - 1))
                    for sl in range(2):
                        w = 2 * hp + sl
                        rsum = work.tile([1, 512], F32, tag="rsum")
                        nc.vector.tensor_copy(rsum[:], avt[sl][64:65, :])
                        rcp = work.tile([1, 512], F32, tag="rcp")
                        nc.vector.reciprocal_approx_fast(rcp[:], rsum[:])
                        bc = work.tile([64, 512], F32, tag="bc")
                        nc.gpsimd.partition_broadcast(bc[:], rcp[0:1, :])
                        p0 = (w % 2) * 64
                        nc.vector.tensor_tensor(
                            aoT[hp][p0:p0 + 64, ts], avt[sl][0:64, :], bc[:],
                            ALU.mult)
                if tb == 1:
                    emit_gather_oproj(0)
                if tb == 3:
                    emit_gather_oproj(1)

            if DEBUG:
                for nm, tl in (("dbg_qt", QT[0]), ("dbg_kt", KT[0]),
                               ("dbg_ao", aoT[0])):
                    dt_f = work.tile([128, T], F32, tag="dbgf", name="dbgf")
                    nc.vector.tensor_copy(dt_f[:], tl[:])
                    nc.sync.dma_start({"dbg_qt": dbg_qt, "dbg_kt": dbg_kt,
                                       "dbg_ao": dbg_ao}[nm][:], dt_f[:])
                dt_f = work.tile([128, 260], F32, tag="dbgv", name="dbgv")
                nc.vector.tensor_copy(dt_f[:], V3[:, 0, :])
                nc.sync.dma_start(dbg_v[:], dt_f[:])
                dg_f = work.tile([128, T], F32, tag="dbgf", name="dbgf")
                nc.vector.tensor_copy(dg_f[:], aog[0][:])
                nc.sync.dma_start(dbg_ag[:], dg_f[:])

            for ocb in range(2):
                nc.sync.dma_start(out_d[128 * ocb:128 * (ocb + 1), :],
                                  outp[ocb][:])
    nc.compile()
    return nc


_NC_CACHE = {}


def get_nc():
    if "nc" not in _NC_CACHE:
        _NC_CACHE["nc"] = _build_nc()
    return _NC_CACHE["nc"]


def make_in_maps(x, qW, qb, kW, kb, vW, vb, oW, ob, J):
    x = np.asarray(x, np.float32)
    Wq, bq = _build_big(qW, qb)
    Wk, bk = _build_big(kW, kb)
    Wv, bv = _build_big(vW, vb)
    Wo, bo = _build_big(oW, ob)
    perm = np.concatenate([_cols_for_group(g) for g in range(NG)])
    Wo_perm = np.ascontiguousarray(Wo[perm, :])
    s0 = (np.arange(128)[:, None] % 64 == np.arange(128)[None, :] % 64)
    s0 = s0.astype(np.float32)
    xts = [np.ascontiguousarray(x[b].T) for b in range(B)]
    in_maps = []
    for c in range(NCORES):
        b, g = divmod(c, NG)
        cols = _cols_for_group(g)
        wv_ext = np.zeros((D, 260), np.float32)
        bv_ext = np.zeros(260, np.float32)
        for w in range(4):
            wv_ext[:, 65 * w:65 * w + 64] = Wv[:, cols[64 * w:64 * w + 64]]
            bv_ext[65 * w:65 * w + 64] = bv[cols[64 * w:64 * w + 64]]
            bv_ext[65 * w + 64] = 1.0
        in_maps.append({
            "xt": xts[b],
            "wq": np.ascontiguousarray(Wq[:, cols]),
            "wk": np.ascontiguousarray(Wk[:, cols]),
            "wv": wv_ext,
            "wo": np.ascontiguousarray(Wo_perm[:, 256 * g:256 * (g + 1)]),
            "bq": np.ascontiguousarray(bq[cols].reshape(2, 128).T),
            "bk": np.ascontiguousarray(bk[cols].reshape(2, 128).T),
            "bv": np.broadcast_to(bv_ext, (128, 260)).copy(),
            "bo": np.ascontiguousarray(
                bo[256 * g:256 * (g + 1)].reshape(2, 128).T),
            "s0": s0,
        })
    return in_maps


def assemble_output(results):
    out = np.zeros((B, T, D), np.float32)
    for c in range(NCORES):
        b, g = divmod(c, NG)
        out[b, :, 256 * g:256 * (g + 1)] = results[c]["out"].T
    return out


def kernel(**inputs):
    in_maps = make_in_maps(**inputs)
    nc = get_nc()
    res = run_bass_kernel_spmd(nc, in_maps, core_ids=list(range(NCORES)))
    return assemble_output(res.results)


if __name__ == "__main__":
    import reference
    inputs = {k: np.asarray(v) for k, v in reference.setup_inputs().items()}
    expected = np.asarray(reference.reference(**inputs))
    actual = kernel(**inputs)
    err = np.linalg.norm(actual - expected) / np.linalg.norm(expected)
    print(f"Relative error: {err:.3e}")
